# revision 6
# baseline (speedup 1.0000x reference)
"""Trainium2 Bass kernel for NeuralODETrajectory.

Math: reference integrates y' = y @ W.T + b with dopri5, 2 fixed substeps of
h = dt/2 per interval, 31 intervals. For b == 0 and uniform dt the dynamics
are linear with a constant per-interval propagator A = S(h)^2 (S = dopri5
step matrix), so y_t = y0 @ A^t.  With E = A - I (spectral norm ~0.02),
(I+E)^t = sum_j binom(t,j) E^j truncates at j<=4 with error ~1e-4 << the
2e-2 tolerance.  The device therefore:

  1. builds the Krylov basis u_j = y0 @ E^j (j=1..4) with 4 GEMMs
     (u1 = y0 E1, u2 = y0 E2, u3 = u2 E1, u4 = u2 E2; E2 = E^2 from host),
  2. relays the basis out via SBUF->SBUF DMA (SWDGE) into a packed layout
     upk[5*s + j, (m - 32 s)*1024 + n] = u_j[m, n]  (4 chunk-slots of 32
     batch rows each on partitions 0..19),
  3. emits all 31 outputs as rank-5 combinations with tiny K=20 matmuls:
     psum[32*s + t-1, q] = sum_j binom(t,j) * upk[5 s + j, col q]
     (the binomial stationary packs 4 chunks x 31 times into one 128-wide
     output), evacuating PSUM->SBUF as bf16 on alternating DVE/ACT into
     16-group stage buffers and DMA-ing ~1 MB blocks straight to HBM.

Per-output cost collapses from one [128,1024]@[1024,1024] GEMM (baseline
chain) to 1/4 of a 512-column matmul, leaving the kernel bound by output
evacuation/DMA instead of TensorE.

Sharding: data-parallel over batch - 128 rows per core; E powers replicated.
"""

import numpy as np

D = 1024
NB = D // 128          # 8 contraction blocks
N_CORES = 8
ROWS = D // N_CORES    # 128 batch rows per core
T = 32
NT = T - 1             # device-produced time slices (t = 1..31)
J = 5                  # basis vectors u_0..u_4
NS = 4                 # chunk slots (batch split per core)
CH = ROWS // NS        # 32 batch rows per chunk
UCOLS = CH * D         # 32768 packed columns per chunk
GBLK = 16              # combination groups staged per output DMA

_CACHE = {}


def _build():
    import concourse.bacc as bacc
    import concourse.mybir as mybir
    from concourse import tile, masks

    f32 = mybir.dt.float32
    bf16 = mybir.dt.bfloat16

    nc = bacc.Bacc("TRN2", target_bir_lowering=False, debug=False,
                   num_devices=N_CORES)
    y0b = nc.dram_tensor("y0b", [ROWS, D], bf16, kind="ExternalInput").ap()
    e1 = nc.dram_tensor("e1", [128, NB * D], bf16, kind="ExternalInput").ap()
    e2 = nc.dram_tensor("e2", [128, NB * D], bf16, kind="ExternalInput").ap()
    u0p = nc.dram_tensor("u0p", [NS, UCOLS], bf16, kind="ExternalInput").ap()
    cm = nc.dram_tensor("cm", [32, 128], bf16, kind="ExternalInput").ap()
    out = nc.dram_tensor("out", [NT, ROWS, D], bf16,
                         kind="ExternalOutput").ap()

    with tile.TileContext(nc) as tc:
        with tc.tile_pool(name="sbuf", bufs=1) as pool, \
             tc.tile_pool(name="psum", bufs=1, space="PSUM") as psum:
            identb = pool.tile([128, 128], bf16, tag="identb")
            masks.make_identity(nc, identb[:])

            e1_sb = pool.tile([128, NB * D], bf16, tag="e1")
            e2_sb = pool.tile([128, NB * D], bf16, tag="e2")
            y0_sb = pool.tile([ROWS, D], bf16, tag="y0")
            cm_sb = pool.tile([32, 128], bf16, tag="cm")
            upk = pool.tile([J * NS, UCOLS], bf16, tag="upk")
            y0T = pool.tile([128, D], bf16, tag="y0T")
            u2T = pool.tile([128, D], bf16, tag="u2T")
            u_sb = [pool.tile([ROWS, D], bf16, tag=f"u{j}", name=f"u{j}")
                    for j in range(1, J)]

            nc.sync.dma_start(out=y0_sb[:], in_=y0b)
            nc.sync.dma_start(out=e1_sb[:], in_=e1)
            nc.gpsimd.dma_start(out=upk[0:J * NS:J, :], in_=u0p)
            nc.gpsimd.dma_start(out=cm_sb[:], in_=cm)
            nc.sync.dma_start(out=e2_sb[:], in_=e2)

            def transpose_to(dst, src):
                # dst[p, 128k + m] = src[m, 128k + p], blockwise via PSUM
                for g in range(2):
                    tp = psum.tile([128, 512], bf16, tag="tp", name=f"tp{g}",
                                   bufs=2)
                    for kk in range(4):
                        k = 4 * g + kk
                        nc.tensor.transpose(tp[:, kk * 128:(kk + 1) * 128],
                                            src[:, k * 128:(k + 1) * 128],
                                            identb[:])
                    nc.scalar.copy(dst[:, g * 512:(g + 1) * 512], tp[:])

            transpose_to(y0T, y0_sb)

            def gemm(dst, lT, rhs_sb):
                # dst = (lT.T) @ E  with E in k-block layout [128, 8*1024]
                for h in range(2):
                    pu = psum.tile([128, 512], f32, tag="pu", name="pu",
                                   bufs=2)
                    for k in range(NB):
                        nc.tensor.matmul(
                            pu[:], lT[:, k * 128:(k + 1) * 128],
                            rhs_sb[:, k * D + h * 512: k * D + h * 512 + 512],
                            start=(k == 0), stop=(k == NB - 1))
                    nc.vector.tensor_copy(dst[:, h * 512:(h + 1) * 512],
                                          pu[:])

            gemm(u_sb[0], y0T, e1_sb)              # u1 = y0 E
            gemm(u_sb[1], y0T, e2_sb)              # u2 = y0 E^2
            transpose_to(u2T, u_sb[1])
            gemm(u_sb[2], u2T, e1_sb)              # u3 = y0 E^3
            gemm(u_sb[3], u2T, e2_sb)              # u4 = y0 E^4

            # SBUF->SBUF relayout: one DMA per (j, chunk) — a [CH, D] block
            # flattens into one 64 KB partition row of upk. (A single DMA
            # with a partition-folding rearrange mis-generates descriptors.)
            for j in range(1, J):
                for s in range(NS):
                    nc.gpsimd.dma_start(
                        out=upk[J * s + j:J * s + j + 1, :],
                        in_=u_sb[j - 1][CH * s:CH * (s + 1), :])

            for b in range(CH // GBLK):
                stage = pool.tile([128, GBLK * D], bf16, tag="stage",
                                  name="stage", bufs=2)
                for gg in range(GBLK):
                    g = GBLK * b + gg
                    pc = psum.tile([128, D], f32, tag="pc", name="pc", bufs=2)
                    for h in range(2):
                        nc.tensor.matmul(
                            pc[:, h * 512:(h + 1) * 512], cm_sb[0:J * NS, :],
                            upk[0:J * NS,
                                (2 * g + h) * 512:(2 * g + h + 1) * 512],
                            start=True, stop=True)
                    dst = stage[:, gg * D:(gg + 1) * D]
                    if gg % 2 == 0:
                        nc.vector.tensor_copy(dst, pc[:])
                    else:
                        nc.scalar.copy(dst, pc[:])
                for s in range(NS):
                    nc.sync.dma_start(
                        out=out[:, CH * s + GBLK * b:CH * s + GBLK * (b + 1), :],
                        in_=stage[CH * s:CH * s + NT, :])

    nc.compile()
    return nc


def _get_nc():
    nc = _CACHE.get("nc")
    if nc is None:
        nc = _build()
        _CACHE["nc"] = nc
    return nc


def _dopri5_step(y, h, M, b):
    def f(v):
        return v @ M + b
    k1 = f(y)
    k2 = f(y + h * (1.0/5.0) * k1)
    k3 = f(y + h * (3.0/40.0*k1 + 9.0/40.0*k2))
    k4 = f(y + h * (44.0/45.0*k1 - 56.0/15.0*k2 + 32.0/9.0*k3))
    k5 = f(y + h * (19372.0/6561.0*k1 - 25360.0/2187.0*k2
                    + 64448.0/6561.0*k3 - 212.0/729.0*k4))
    k6 = f(y + h * (9017.0/3168.0*k1 - 355.0/33.0*k2 + 46732.0/5247.0*k3
                    + 49.0/176.0*k4 - 5103.0/18656.0*k5))
    return y + h * (35.0/384.0*k1 + 500.0/1113.0*k3 + 125.0/192.0*k4
                    - 2187.0/6784.0*k5 + 11.0/84.0*k6)


def _host_mats(W32, dt):
    """E1 = A - I, E2 = E1^2 for the interval propagator A (f64)."""
    M = W32.T.astype(np.float64)
    S = _dopri5_step(np.eye(D), dt / 2.0, M, 0.0)
    A = S @ S
    E1 = A - np.eye(D)
    E2 = E1 @ E1
    return E1, E2


def _binom_stationary():
    from math import comb
    C = np.zeros((32, 128), dtype=np.float64)
    for s in range(NS):
        for j in range(J):
            for tau in range(NT):
                C[J * s + j, 32 * s + tau] = comb(tau + 1, j)
    return C


def _fallback(start_embedding, t_eval, W, b):
    M = W.T.astype(np.float64)
    bb = np.asarray(b, dtype=np.float64)
    y = start_embedding.astype(np.float64)
    t = np.asarray(t_eval, dtype=np.float64)
    traj = [y.copy()]
    for k in range(t.shape[0] - 1):
        h = (t[k+1] - t[k]) / 2.0
        for _ in range(2):
            y = _dopri5_step(y, h, M, bb)
        traj.append(y.copy())
    return np.stack(traj).astype(np.float32)


def _kblock(E, bf16):
    # [1024,1024] -> [128, 8*1024] with E_kb[p, 1024 k + n] = E[128 k + p, n]
    return np.ascontiguousarray(
        E.reshape(NB, 128, D).transpose(1, 0, 2).reshape(128, NB * D)
    ).astype(bf16)


def _make_in_maps(y0, t_eval=None, W=None):
    import ml_dtypes
    bf16 = ml_dtypes.bfloat16
    dt = 1.0 if t_eval is None else float(np.asarray(t_eval)[1]
                                          - np.asarray(t_eval)[0])
    E1, E2 = _host_mats(W, dt)
    e1 = _kblock(E1, bf16)
    e2 = _kblock(E2, bf16)
    cmat = _binom_stationary().astype(bf16)
    maps = []
    for c in range(N_CORES):
        y0c = np.ascontiguousarray(y0[c * ROWS:(c + 1) * ROWS, :]).astype(bf16)
        u0c = np.ascontiguousarray(y0c.reshape(NS, UCOLS))
        maps.append({"y0b": y0c, "e1": e1, "e2": e2, "u0p": u0c, "cm": cmat})
    return maps


def _assemble(y0, results):
    out = np.empty((T, D, D), dtype=np.float32)
    out[0] = y0
    for c in range(N_CORES):
        dev = results[c]["out"].astype(np.float32)      # [31, 128, 1024]
        out[1:, c * ROWS:(c + 1) * ROWS, :] = dev
    return out


def kernel(start_embedding, t_eval, W, b):
    start_embedding = np.ascontiguousarray(start_embedding, dtype=np.float32)
    W32 = np.ascontiguousarray(W, dtype=np.float32)
    t = np.asarray(t_eval, dtype=np.float64)
    dts = np.diff(t)
    fast_ok = (start_embedding.shape == (D, D) and W32.shape == (D, D)
               and t.shape == (T,) and dts.size > 0
               and np.all(np.abs(dts - dts[0]) <= 1e-12 * abs(dts[0]))
               and not np.any(np.asarray(b)))
    if not fast_ok:
        return _fallback(start_embedding, t_eval, W32, np.asarray(b))

    from concourse.bass_utils import run_bass_kernel_spmd
    nc = _get_nc()
    in_maps = _make_in_maps(start_embedding, t, W32)
    res = run_bass_kernel_spmd(nc, in_maps, list(range(N_CORES)))
    return _assemble(start_embedding, res.results)


# revision 15
# speedup vs baseline: 1.2689x; 1.2689x over previous
"""Trainium2 Bass kernel for NeuralODETrajectory.

Math: reference integrates y' = y @ W.T + b with dopri5, 2 fixed substeps of
h = dt/2 per interval, 31 intervals. For b == 0 and uniform dt the dynamics
are linear with a constant per-interval propagator A = S(h)^2 (S = dopri5
step matrix), so y_t = y0 @ A^t.  With E = A - I (spectral norm ~0.02),
(I+E)^t = sum_j binom(t,j) E^j truncates at j<=4 with error ~1e-4 << the
2e-2 tolerance.  The device therefore:

  1. builds the Krylov basis u_j = y0 @ E^j (j=1..4) with 4 GEMMs
     (u1 = y0 E1, u2 = y0 E2, u3 = u2 E1, u4 = u2 E2; E2 = E^2 from host),
  2. relays the basis out via SBUF->SBUF DMA into a packed layout
     upk[5*s + j, (m - 32 s)*1024 + n] = u_j[m, n]  (4 chunk-slots of 32
     batch rows each on partitions 0..19),
  3. emits all 31 outputs as rank-5 combinations with tiny K=20 matmuls:
     psum[32*s + t-1, q] = sum_j binom(t,j) * upk[5 s + j, col q]
     (the binomial stationary packs 4 chunks x 31 times into one 128-wide
     output), evacuating PSUM->SBUF as bf16 on alternating DVE/ACT engines
     into 8-group stage buffers and DMA-ing ~0.5 MB blocks straight to HBM.

Per-output cost collapses from one [128,1024]@[1024,1024] GEMM (baseline
chain) to 1/4 of a 512-column matmul, leaving the kernel bound by output
evacuation/DMA instead of TensorE.

Sharding: data-parallel over batch - 128 rows per core; E powers replicated.
"""

import numpy as np

D = 1024
NB = D // 128          # 8 contraction blocks
N_CORES = 8
ROWS = D // N_CORES    # 128 batch rows per core
T = 32
NT = T - 1             # device-produced time slices (t = 1..31)
J = 5                  # basis vectors u_0..u_4
NS = 4                 # chunk slots (batch split per core)
CH = ROWS // NS        # 32 batch rows per chunk
UCOLS = CH * D         # 32768 packed columns per chunk
GBLK = 8               # combination groups staged per output DMA

_CACHE = {}


def _build():
    import concourse.bacc as bacc
    import concourse.mybir as mybir
    from concourse import tile, masks

    f32 = mybir.dt.float32
    bf16 = mybir.dt.bfloat16

    nc = bacc.Bacc("TRN2", target_bir_lowering=False, debug=False,
                   num_devices=N_CORES)
    y0b = nc.dram_tensor("y0b", [ROWS, D], bf16, kind="ExternalInput").ap()
    e1 = nc.dram_tensor("e1", [128, NB * D], bf16, kind="ExternalInput").ap()
    e2 = nc.dram_tensor("e2", [128, NB * D], bf16, kind="ExternalInput").ap()
    u0p = nc.dram_tensor("u0p", [NS, UCOLS], bf16, kind="ExternalInput").ap()
    cm = nc.dram_tensor("cm", [32, 128], bf16, kind="ExternalInput").ap()
    out = nc.dram_tensor("out", [NT, ROWS, D], bf16,
                         kind="ExternalOutput").ap()

    with tile.TileContext(nc) as tc:
        with tc.tile_pool(name="sbuf", bufs=1) as pool, \
             tc.tile_pool(name="psum", bufs=1, space="PSUM") as psum:
            identb = pool.tile([128, 128], bf16, tag="identb")
            masks.make_identity(nc, identb[:])

            e1_sb = pool.tile([128, NB * D], bf16, tag="e1")
            e2_sb = pool.tile([128, NB * D], bf16, tag="e2")
            y0_sb = pool.tile([ROWS, D], bf16, tag="y0")
            cm_sb = pool.tile([32, 128], bf16, tag="cm")
            upk = pool.tile([J * NS, UCOLS], bf16, tag="upk")
            y0T = pool.tile([128, D], bf16, tag="y0T")
            u2T = pool.tile([128, D], bf16, tag="u2T")
            u_sb = [pool.tile([ROWS, D], bf16, tag=f"u{j}", name=f"u{j}")
                    for j in range(1, J)]

            nc.sync.dma_start(out=y0_sb[:], in_=y0b)
            half = NB * D // 2
            nc.sync.dma_start(out=e1_sb[:, 0:half], in_=e1[:, 0:half])
            nc.sync.dma_start(out=e1_sb[:, half:], in_=e1[:, half:])
            nc.sync.dma_start(out=e2_sb[:, 0:half], in_=e2[:, 0:half])
            nc.sync.dma_start(out=e2_sb[:, half:], in_=e2[:, half:])
            nc.sync.dma_start(out=upk[0:J * NS:J, :], in_=u0p)
            nc.sync.dma_start(out=cm_sb[:], in_=cm)

            def transpose_to(dst, src):
                # dst[p, 128k + m] = src[m, 128k + p], blockwise via PSUM
                for g in range(2):
                    tp = psum.tile([128, 512], bf16, tag="tp", name=f"tp{g}",
                                   bufs=2)
                    for kk in range(4):
                        k = 4 * g + kk
                        nc.tensor.transpose(tp[:, kk * 128:(kk + 1) * 128],
                                            src[:, k * 128:(k + 1) * 128],
                                            identb[:])
                    if g == 0:
                        nc.scalar.copy(dst[:, g * 512:(g + 1) * 512], tp[:])
                    else:
                        nc.vector.tensor_copy(dst[:, g * 512:(g + 1) * 512],
                                              tp[:])

            transpose_to(y0T, y0_sb)

            def gemm(dst, lT, rhs_sb):
                # dst = (lT.T) @ E  with E in k-block layout [128, 8*1024]
                for h in range(2):
                    pu = psum.tile([128, 512], f32, tag="pu", name="pu",
                                   bufs=2)
                    for k in range(NB):
                        nc.tensor.matmul(
                            pu[:], lT[:, k * 128:(k + 1) * 128],
                            rhs_sb[:, k * D + h * 512: k * D + h * 512 + 512],
                            start=(k == 0), stop=(k == NB - 1))
                    if h == 0:
                        nc.vector.tensor_copy(dst[:, h * 512:(h + 1) * 512],
                                              pu[:])
                    else:
                        nc.scalar.copy(dst[:, h * 512:(h + 1) * 512], pu[:])

            # SBUF->SBUF relayout: one DMA per (j, chunk) — a [CH, D] block
            # flattens into one 64 KB partition row of upk. (A single DMA
            # with a partition-folding rearrange mis-generates descriptors.)
            def relayout(j):
                for s in range(NS):
                    nc.sync.dma_start(
                        out=upk[J * s + j:J * s + j + 1, :],
                        in_=u_sb[j - 1][CH * s:CH * (s + 1), :])

            u1T = pool.tile([128, D], bf16, tag="u1T")
            gemm(u_sb[0], y0T, e1_sb)              # u1 = y0 E
            gemm(u_sb[1], y0T, e2_sb)              # u2 = y0 E^2
            relayout(1)
            transpose_to(u1T, u_sb[0])
            relayout(2)
            gemm(u_sb[2], u1T, e2_sb)              # u3 = y0 E^3
            transpose_to(u2T, u_sb[1])
            relayout(3)
            gemm(u_sb[3], u2T, e2_sb)              # u4 = y0 E^4
            relayout(4)

            g0 = 0
            for blk in (8, 8, 8, 4, 4):
                stage = pool.tile([128, GBLK * D], bf16, tag="stage",
                                  name="stage", bufs=2)
                for gg in range(blk):
                    g = g0 + gg
                    for h in range(2):
                        pc = psum.tile([128, 512], f32, tag="pc", name="pc",
                                       bufs=4)
                        nc.tensor.matmul(
                            pc[:], cm_sb[0:J * NS, :],
                            upk[0:J * NS,
                                (2 * g + h) * 512:(2 * g + h + 1) * 512],
                            start=True, stop=True)
                        dst = stage[:, gg * D + h * 512:gg * D + h * 512 + 512]
                        if h == 0:
                            nc.vector.tensor_copy(dst, pc[:])
                        else:
                            nc.scalar.copy(dst, pc[:])
                for s in range(NS):
                    nc.sync.dma_start(
                        out=out[:, CH * s + g0:CH * s + g0 + blk, :],
                        in_=stage[CH * s:CH * s + NT, 0:blk * D])
                g0 += blk

    nc.compile()
    return nc


def _get_nc():
    nc = _CACHE.get("nc")
    if nc is None:
        nc = _build()
        _CACHE["nc"] = nc
    return nc


def _dopri5_step(y, h, M, b):
    def f(v):
        return v @ M + b
    k1 = f(y)
    k2 = f(y + h * (1.0/5.0) * k1)
    k3 = f(y + h * (3.0/40.0*k1 + 9.0/40.0*k2))
    k4 = f(y + h * (44.0/45.0*k1 - 56.0/15.0*k2 + 32.0/9.0*k3))
    k5 = f(y + h * (19372.0/6561.0*k1 - 25360.0/2187.0*k2
                    + 64448.0/6561.0*k3 - 212.0/729.0*k4))
    k6 = f(y + h * (9017.0/3168.0*k1 - 355.0/33.0*k2 + 46732.0/5247.0*k3
                    + 49.0/176.0*k4 - 5103.0/18656.0*k5))
    return y + h * (35.0/384.0*k1 + 500.0/1113.0*k3 + 125.0/192.0*k4
                    - 2187.0/6784.0*k5 + 11.0/84.0*k6)


def _host_mats(W32, dt):
    """E1 = A - I, E2 = E1^2 for the interval propagator A (f64)."""
    M = W32.T.astype(np.float64)
    S = _dopri5_step(np.eye(D), dt / 2.0, M, 0.0)
    A = S @ S
    E1 = A - np.eye(D)
    E2 = E1 @ E1
    return E1, E2


def _binom_stationary():
    from math import comb
    C = np.zeros((32, 128), dtype=np.float64)
    for s in range(NS):
        for j in range(J):
            for tau in range(NT):
                C[J * s + j, 32 * s + tau] = comb(tau + 1, j)
    return C


def _fallback(start_embedding, t_eval, W, b):
    M = W.T.astype(np.float64)
    bb = np.asarray(b, dtype=np.float64)
    y = start_embedding.astype(np.float64)
    t = np.asarray(t_eval, dtype=np.float64)
    traj = [y.copy()]
    for k in range(t.shape[0] - 1):
        h = (t[k+1] - t[k]) / 2.0
        for _ in range(2):
            y = _dopri5_step(y, h, M, bb)
        traj.append(y.copy())
    return np.stack(traj).astype(np.float32)


def _kblock(E, bf16):
    # [1024,1024] -> [128, 8*1024] with E_kb[p, 1024 k + n] = E[128 k + p, n]
    return np.ascontiguousarray(
        E.reshape(NB, 128, D).transpose(1, 0, 2).reshape(128, NB * D)
    ).astype(bf16)


def _make_in_maps(y0, t_eval=None, W=None):
    import ml_dtypes
    bf16 = ml_dtypes.bfloat16
    dt = 1.0 if t_eval is None else float(np.asarray(t_eval)[1]
                                          - np.asarray(t_eval)[0])
    E1, E2 = _host_mats(W, dt)
    e1 = _kblock(E1, bf16)
    e2 = _kblock(E2, bf16)
    cmat = _binom_stationary().astype(bf16)
    maps = []
    for c in range(N_CORES):
        y0c = np.ascontiguousarray(y0[c * ROWS:(c + 1) * ROWS, :]).astype(bf16)
        u0c = np.ascontiguousarray(y0c.reshape(NS, UCOLS))
        maps.append({"y0b": y0c, "e1": e1, "e2": e2, "u0p": u0c, "cm": cmat})
    return maps


def _assemble(y0, results):
    out = np.empty((T, D, D), dtype=np.float32)
    out[0] = y0
    for c in range(N_CORES):
        dev = results[c]["out"].astype(np.float32)      # [31, 128, 1024]
        out[1:, c * ROWS:(c + 1) * ROWS, :] = dev
    return out


def kernel(start_embedding, t_eval, W, b):
    start_embedding = np.ascontiguousarray(start_embedding, dtype=np.float32)
    W32 = np.ascontiguousarray(W, dtype=np.float32)
    t = np.asarray(t_eval, dtype=np.float64)
    dts = np.diff(t)
    fast_ok = (start_embedding.shape == (D, D) and W32.shape == (D, D)
               and t.shape == (T,) and dts.size > 0
               and np.all(np.abs(dts - dts[0]) <= 1e-12 * abs(dts[0]))
               and not np.any(np.asarray(b)))
    if not fast_ok:
        return _fallback(start_embedding, t_eval, W32, np.asarray(b))

    from concourse.bass_utils import run_bass_kernel_spmd
    nc = _get_nc()
    in_maps = _make_in_maps(start_embedding, t, W32)
    res = run_bass_kernel_spmd(nc, in_maps, list(range(N_CORES)))
    return _assemble(start_embedding, res.results)


# revision 27
# speedup vs baseline: 1.4006x; 1.1038x over previous
"""Trainium2 Bass kernel for NeuralODETrajectory.

Math: reference integrates y' = y @ W.T + b with dopri5, 2 fixed substeps of
h = dt/2 per interval, 31 intervals. For b == 0 and uniform dt the dynamics
are linear with a constant per-interval propagator A = S(h)^2 (S = dopri5
step matrix), so y_t = y0 @ A^t.  With E = A - I (spectral norm ~0.02),
(I+E)^t = sum_j binom(t,j) E^j truncates at j<=4 with error ~1e-4 << the
2e-2 tolerance.  The device therefore:

  1. builds the Krylov basis u_j = y0 @ E^j (j=1..4) with 4 GEMMs
     (u1 = y0 E1, u2 = y0 E2, u3 = u2 E1, u4 = u2 E2; E2 = E^2 from host),
  2. relays the basis out via SBUF->SBUF DMA into a packed layout
     upk[5*s + j, (m - 32 s)*1024 + n] = u_j[m, n]  (4 chunk-slots of 32
     batch rows each on partitions 0..19),
  3. emits all 31 outputs as rank-5 combinations with tiny K=20 matmuls:
     psum[32*s + t-1, q] = sum_j binom(t,j) * upk[5 s + j, col q]
     (the binomial stationary packs 4 chunks x 31 times into one 128-wide
     output), evacuating PSUM->SBUF as bf16 on alternating DVE/ACT engines
     into 8-group stage buffers and DMA-ing ~0.5 MB blocks straight to HBM.

Per-output cost collapses from one [128,1024]@[1024,1024] GEMM (baseline
chain) to 1/4 of a 512-column matmul, leaving the kernel bound by output
evacuation/DMA instead of TensorE.

Sharding: data-parallel over batch - 128 rows per core; E powers replicated.
"""

import numpy as np

D = 1024
NB = D // 128          # 8 contraction blocks
N_CORES = 8
ROWS = D // N_CORES    # 128 batch rows per core
T = 32
NT = T - 1             # device-produced time slices (t = 1..31)
J = 5                  # basis vectors u_0..u_4
NS = 4                 # chunk slots (batch split per core)
CH = ROWS // NS        # 32 batch rows per chunk
UCOLS = CH * D         # 32768 packed columns per chunk
GBLK = 8               # combination groups staged per output DMA

_CACHE = {}


def _build():
    import concourse.bacc as bacc
    import concourse.mybir as mybir
    from concourse import tile, masks

    f32 = mybir.dt.float32
    bf16 = mybir.dt.bfloat16

    nc = bacc.Bacc("TRN2", target_bir_lowering=False, debug=False,
                   num_devices=N_CORES)
    y0b = nc.dram_tensor("y0b", [ROWS, D], bf16, kind="ExternalInput").ap()
    e1 = nc.dram_tensor("e1", [128, NB * D], bf16, kind="ExternalInput").ap()
    e2 = nc.dram_tensor("e2", [128, NB * D], bf16, kind="ExternalInput").ap()
    e3 = nc.dram_tensor("e3", [128, NB * D], bf16, kind="ExternalInput").ap()
    u0p = nc.dram_tensor("u0p", [NS, UCOLS], bf16, kind="ExternalInput").ap()
    cm = nc.dram_tensor("cm", [32, 128], bf16, kind="ExternalInput").ap()
    out = nc.dram_tensor("out", [NT, ROWS, D], bf16,
                         kind="ExternalOutput").ap()

    with tile.TileContext(nc) as tc:
        with tc.tile_pool(name="sbuf", bufs=1) as pool, \
             tc.tile_pool(name="psum", bufs=1, space="PSUM") as psum:
            identb = pool.tile([128, 128], bf16, tag="identb")
            masks.make_identity(nc, identb[:])

            e1_sb = pool.tile([128, NB * D], bf16, tag="e1")
            e2_sb = pool.tile([128, NB * D], bf16, tag="e2")
            e3_sb = pool.tile([128, NB * D], bf16, tag="e3")
            y0_sb = pool.tile([ROWS, D], bf16, tag="y0")
            cm_sb = pool.tile([32, 128], bf16, tag="cm")
            upk = pool.tile([J * NS, UCOLS], bf16, tag="upk")
            y0T = pool.tile([128, D], bf16, tag="y0T")
            u_sb = [pool.tile([ROWS, D], bf16, tag=f"u{j}", name=f"u{j}")
                    for j in range(1, J)]

            nc.sync.dma_start(out=y0_sb[:], in_=y0b)
            half = NB * D // 2
            nc.sync.dma_start(out=e1_sb[:, 0:half], in_=e1[:, 0:half])
            nc.sync.dma_start(out=e1_sb[:, half:], in_=e1[:, half:])
            nc.sync.dma_start(out=e2_sb[:, 0:half], in_=e2[:, 0:half])
            nc.sync.dma_start(out=e2_sb[:, half:], in_=e2[:, half:])
            nc.sync.dma_start(out=e3_sb[:, 0:half], in_=e3[:, 0:half])
            nc.sync.dma_start(out=e3_sb[:, half:], in_=e3[:, half:])
            nc.sync.dma_start(out=upk[0:J * NS:J, :], in_=u0p)
            nc.sync.dma_start(out=cm_sb[:], in_=cm)

            def transpose_to(dst, src):
                # dst[p, 128k + m] = src[m, 128k + p], blockwise via PSUM
                for g in range(2):
                    tp = psum.tile([128, 512], bf16, tag="tp", name=f"tp{g}",
                                   bufs=2)
                    for kk in range(4):
                        k = 4 * g + kk
                        nc.tensor.transpose(tp[:, kk * 128:(kk + 1) * 128],
                                            src[:, k * 128:(k + 1) * 128],
                                            identb[:])
                    if g == 0:
                        nc.scalar.copy(dst[:, g * 512:(g + 1) * 512], tp[:])
                    else:
                        nc.vector.tensor_copy(dst[:, g * 512:(g + 1) * 512],
                                              tp[:])

            transpose_to(y0T, y0_sb)

            def gemm(dst, lT, rhs_sb):
                # dst = (lT.T) @ E  with E in k-block layout [128, 8*1024]
                for h in range(2):
                    pu = psum.tile([128, 512], f32, tag="pu", name="pu",
                                   bufs=2)
                    for k in range(NB):
                        nc.tensor.matmul(
                            pu[:], lT[:, k * 128:(k + 1) * 128],
                            rhs_sb[:, k * D + h * 512: k * D + h * 512 + 512],
                            start=(k == 0), stop=(k == NB - 1))
                    if h == 0:
                        nc.vector.tensor_copy(dst[:, h * 512:(h + 1) * 512],
                                              pu[:])
                    else:
                        nc.scalar.copy(dst[:, h * 512:(h + 1) * 512], pu[:])

            # SBUF->SBUF relayout: one DMA per (j, chunk) — a [CH, D] block
            # flattens into one 64 KB partition row of upk. (A single DMA
            # with a partition-folding rearrange mis-generates descriptors.)
            def relayout(j):
                for s in range(NS):
                    nc.sync.dma_start(
                        out=upk[J * s + j:J * s + j + 1, :],
                        in_=u_sb[j - 1][CH * s:CH * (s + 1), :])

            u1T = pool.tile([128, D], bf16, tag="u1T")
            gemm(u_sb[0], y0T, e1_sb)              # u1 = y0 E
            gemm(u_sb[1], y0T, e2_sb)              # u2 = y0 E^2
            relayout(1)
            transpose_to(u1T, u_sb[0])
            relayout(2)
            gemm(u_sb[2], u1T, e2_sb)              # u3 = y0 E^3
            relayout(3)
            gemm(u_sb[3], u1T, e3_sb)              # u4 = y0 E^4
            relayout(4)

            g0 = 0
            for blk in (2, 2, 8, 8, 8, 2, 2):
                stage = pool.tile([128, GBLK * D], bf16, tag="stage",
                                  name="stage", bufs=4)
                for gg in range(blk):
                    g = g0 + gg
                    for h in range(2):
                        pc = psum.tile([128, 512], f32, tag="pc", name="pc",
                                       bufs=4)
                        nc.tensor.matmul(
                            pc[:], cm_sb[0:J * NS, :],
                            upk[0:J * NS,
                                (2 * g + h) * 512:(2 * g + h + 1) * 512],
                            start=True, stop=True)
                        dst = stage[:, gg * D + h * 512:gg * D + h * 512 + 512]
                        if h == 0:
                            nc.vector.tensor_copy(dst, pc[:])
                        else:
                            nc.scalar.copy(dst, pc[:])
                for s in range(NS):
                    nc.sync.dma_start(
                        out=out[:, CH * s + g0:CH * s + g0 + blk, :],
                        in_=stage[CH * s:CH * s + NT, 0:blk * D])
                g0 += blk

    nc.compile()
    return nc


def _get_nc():
    nc = _CACHE.get("nc")
    if nc is None:
        nc = _build()
        _CACHE["nc"] = nc
    return nc


def _dopri5_step(y, h, M, b):
    def f(v):
        return v @ M + b
    k1 = f(y)
    k2 = f(y + h * (1.0/5.0) * k1)
    k3 = f(y + h * (3.0/40.0*k1 + 9.0/40.0*k2))
    k4 = f(y + h * (44.0/45.0*k1 - 56.0/15.0*k2 + 32.0/9.0*k3))
    k5 = f(y + h * (19372.0/6561.0*k1 - 25360.0/2187.0*k2
                    + 64448.0/6561.0*k3 - 212.0/729.0*k4))
    k6 = f(y + h * (9017.0/3168.0*k1 - 355.0/33.0*k2 + 46732.0/5247.0*k3
                    + 49.0/176.0*k4 - 5103.0/18656.0*k5))
    return y + h * (35.0/384.0*k1 + 500.0/1113.0*k3 + 125.0/192.0*k4
                    - 2187.0/6784.0*k5 + 11.0/84.0*k6)


def _host_mats(W32, dt):
    """E1 = A - I, E2 = E1^2 for the interval propagator A (f64)."""
    M = W32.T.astype(np.float64)
    S = _dopri5_step(np.eye(D), dt / 2.0, M, 0.0)
    A = S @ S
    E1 = A - np.eye(D)
    E2 = E1 @ E1
    E3 = E2 @ E1
    return E1, E2, E3


def _binom_stationary():
    from math import comb
    C = np.zeros((32, 128), dtype=np.float64)
    for s in range(NS):
        for j in range(J):
            for tau in range(NT):
                C[J * s + j, 32 * s + tau] = comb(tau + 1, j)
    return C


def _fallback(start_embedding, t_eval, W, b):
    M = W.T.astype(np.float64)
    bb = np.asarray(b, dtype=np.float64)
    y = start_embedding.astype(np.float64)
    t = np.asarray(t_eval, dtype=np.float64)
    traj = [y.copy()]
    for k in range(t.shape[0] - 1):
        h = (t[k+1] - t[k]) / 2.0
        for _ in range(2):
            y = _dopri5_step(y, h, M, bb)
        traj.append(y.copy())
    return np.stack(traj).astype(np.float32)


def _kblock(E, bf16):
    # [1024,1024] -> [128, 8*1024] with E_kb[p, 1024 k + n] = E[128 k + p, n]
    return np.ascontiguousarray(
        E.reshape(NB, 128, D).transpose(1, 0, 2).reshape(128, NB * D)
    ).astype(bf16)


def _make_in_maps(y0, t_eval=None, W=None):
    import ml_dtypes
    bf16 = ml_dtypes.bfloat16
    dt = 1.0 if t_eval is None else float(np.asarray(t_eval)[1]
                                          - np.asarray(t_eval)[0])
    E1, E2, E3 = _host_mats(W, dt)
    e1 = _kblock(E1, bf16)
    e2 = _kblock(E2, bf16)
    e3 = _kblock(E3, bf16)
    cmat = _binom_stationary().astype(bf16)
    maps = []
    for c in range(N_CORES):
        y0c = np.ascontiguousarray(y0[c * ROWS:(c + 1) * ROWS, :]).astype(bf16)
        u0c = np.ascontiguousarray(y0c.reshape(NS, UCOLS))
        maps.append({"y0b": y0c, "e1": e1, "e2": e2, "e3": e3, "u0p": u0c, "cm": cmat})
    return maps


def _assemble(y0, results):
    out = np.empty((T, D, D), dtype=np.float32)
    out[0] = y0
    for c in range(N_CORES):
        dev = results[c]["out"].astype(np.float32)      # [31, 128, 1024]
        out[1:, c * ROWS:(c + 1) * ROWS, :] = dev
    return out


def kernel(start_embedding, t_eval, W, b):
    start_embedding = np.ascontiguousarray(start_embedding, dtype=np.float32)
    W32 = np.ascontiguousarray(W, dtype=np.float32)
    t = np.asarray(t_eval, dtype=np.float64)
    dts = np.diff(t)
    fast_ok = (start_embedding.shape == (D, D) and W32.shape == (D, D)
               and t.shape == (T,) and dts.size > 0
               and np.all(np.abs(dts - dts[0]) <= 1e-12 * abs(dts[0]))
               and not np.any(np.asarray(b)))
    if not fast_ok:
        return _fallback(start_embedding, t_eval, W32, np.asarray(b))

    from concourse.bass_utils import run_bass_kernel_spmd
    nc = _get_nc()
    in_maps = _make_in_maps(start_embedding, t, W32)
    res = run_bass_kernel_spmd(nc, in_maps, list(range(N_CORES)))
    return _assemble(start_embedding, res.results)


# revision 29
# speedup vs baseline: 1.4060x; 1.0038x over previous
"""Trainium2 Bass kernel for NeuralODETrajectory.

Math: reference integrates y' = y @ W.T + b with dopri5, 2 fixed substeps of
h = dt/2 per interval, 31 intervals. For b == 0 and uniform dt the dynamics
are linear with a constant per-interval propagator A = S(h)^2 (S = dopri5
step matrix), so y_t = y0 @ A^t.  With E = A - I (spectral norm ~0.02),
(I+E)^t = sum_j binom(t,j) E^j truncates at j<=4 with error ~1e-4 << the
2e-2 tolerance.  The device therefore:

  1. builds the Krylov basis u_j = y0 @ E^j (j=1..4) with 4 GEMMs
     (u1 = y0 E1, u2 = y0 E2, u3 = u2 E1, u4 = u2 E2; E2 = E^2 from host),
  2. relays the basis out via SBUF->SBUF DMA into a packed layout
     upk[5*s + j, (m - 32 s)*1024 + n] = u_j[m, n]  (4 chunk-slots of 32
     batch rows each on partitions 0..19),
  3. emits all 31 outputs as rank-5 combinations with tiny K=20 matmuls:
     psum[32*s + t-1, q] = sum_j binom(t,j) * upk[5 s + j, col q]
     (the binomial stationary packs 4 chunks x 31 times into one 128-wide
     output), evacuating PSUM->SBUF as bf16 on alternating DVE/ACT engines
     into 8-group stage buffers and DMA-ing ~0.5 MB blocks straight to HBM.

Per-output cost collapses from one [128,1024]@[1024,1024] GEMM (baseline
chain) to 1/4 of a 512-column matmul, leaving the kernel bound by output
evacuation/DMA instead of TensorE.

Sharding: data-parallel over batch - 128 rows per core; E powers replicated.
"""

import numpy as np

D = 1024
NB = D // 128          # 8 contraction blocks
N_CORES = 8
ROWS = D // N_CORES    # 128 batch rows per core
T = 32
NT = T - 1             # device-produced time slices (t = 1..31)
J = 5                  # basis vectors u_0..u_4
NS = 4                 # chunk slots (batch split per core)
CH = ROWS // NS        # 32 batch rows per chunk
UCOLS = CH * D         # 32768 packed columns per chunk
GBLK = 8               # combination groups staged per output DMA

_CACHE = {}


def _build():
    import concourse.bacc as bacc
    import concourse.mybir as mybir
    from concourse import tile, masks

    f32 = mybir.dt.float32
    bf16 = mybir.dt.bfloat16
    f8 = mybir.dt.float8e4

    nc = bacc.Bacc("TRN2", target_bir_lowering=False, debug=False,
                   num_devices=N_CORES)
    y0b = nc.dram_tensor("y0b", [ROWS, D], bf16, kind="ExternalInput").ap()
    e1 = nc.dram_tensor("e1", [128, NB * D], bf16, kind="ExternalInput").ap()
    e2 = nc.dram_tensor("e2", [128, NB * D], f8, kind="ExternalInput").ap()
    e3 = nc.dram_tensor("e3", [128, NB * D], f8, kind="ExternalInput").ap()
    u0p = nc.dram_tensor("u0p", [NS, UCOLS], bf16, kind="ExternalInput").ap()
    cm = nc.dram_tensor("cm", [32, 128], bf16, kind="ExternalInput").ap()
    out = nc.dram_tensor("out", [NT, ROWS, D], bf16,
                         kind="ExternalOutput").ap()

    with tile.TileContext(nc) as tc:
        with tc.tile_pool(name="sbuf", bufs=1) as pool, \
             tc.tile_pool(name="psum", bufs=1, space="PSUM") as psum:
            identb = pool.tile([128, 128], bf16, tag="identb")
            masks.make_identity(nc, identb[:])

            e1_sb = pool.tile([128, NB * D], bf16, tag="e1")
            e2_sb = pool.tile([128, NB * D], f8, tag="e2")
            e3_sb = pool.tile([128, NB * D], f8, tag="e3")
            y0_sb = pool.tile([ROWS, D], bf16, tag="y0")
            cm_sb = pool.tile([32, 128], bf16, tag="cm")
            upk = pool.tile([J * NS, UCOLS], bf16, tag="upk")
            y0T = pool.tile([128, D], bf16, tag="y0T")
            y0T8 = pool.tile([128, D], f8, tag="y0T8")
            u_sb = [pool.tile([ROWS, D], bf16, tag=f"u{j}", name=f"u{j}")
                    for j in range(1, J)]

            nc.sync.dma_start(out=y0_sb[:], in_=y0b)
            half = NB * D // 2
            nc.sync.dma_start(out=e1_sb[:, 0:half], in_=e1[:, 0:half])
            nc.sync.dma_start(out=e1_sb[:, half:], in_=e1[:, half:])
            nc.sync.dma_start(out=e2_sb[:, 0:half], in_=e2[:, 0:half])
            nc.sync.dma_start(out=e2_sb[:, half:], in_=e2[:, half:])
            nc.sync.dma_start(out=e3_sb[:, 0:half], in_=e3[:, 0:half])
            nc.sync.dma_start(out=e3_sb[:, half:], in_=e3[:, half:])
            nc.sync.dma_start(out=upk[0:J * NS:J, :], in_=u0p)
            nc.sync.dma_start(out=cm_sb[:], in_=cm)

            def transpose_to(dst, src, dst8=None, scale8=1.0):
                # dst[p, 128k + m] = src[m, 128k + p], blockwise via PSUM
                for g in range(2):
                    tp = psum.tile([128, 512], bf16, tag="tp", name=f"tp{g}",
                                   bufs=2)
                    for kk in range(4):
                        k = 4 * g + kk
                        nc.tensor.transpose(tp[:, kk * 128:(kk + 1) * 128],
                                            src[:, k * 128:(k + 1) * 128],
                                            identb[:])
                    sl = slice(g * 512, (g + 1) * 512)
                    if dst is not None:
                        if g == 0:
                            nc.scalar.copy(dst[:, sl], tp[:])
                        else:
                            nc.vector.tensor_copy(dst[:, sl], tp[:])
                    if dst8 is not None:
                        if g == 0:
                            nc.vector.tensor_scalar_mul(dst8[:, sl], tp[:],
                                                        scale8)
                        else:
                            nc.scalar.mul(dst8[:, sl], tp[:], scale8)

            transpose_to(y0T, y0_sb, dst8=y0T8)

            def gemm(dst, lT, rhs_sb):
                # dst = (lT.T) @ E  with E in k-block layout [128, 8*1024]
                for h in range(2):
                    pu = psum.tile([128, 512], f32, tag="pu", name="pu",
                                   bufs=2)
                    for k in range(NB):
                        nc.tensor.matmul(
                            pu[:], lT[:, k * 128:(k + 1) * 128],
                            rhs_sb[:, k * D + h * 512: k * D + h * 512 + 512],
                            start=(k == 0), stop=(k == NB - 1))
                    if h == 0:
                        nc.vector.tensor_copy(dst[:, h * 512:(h + 1) * 512],
                                              pu[:])
                    else:
                        nc.scalar.copy(dst[:, h * 512:(h + 1) * 512], pu[:])

            # SBUF->SBUF relayout: one DMA per (j, chunk) — a [CH, D] block
            # flattens into one 64 KB partition row of upk. (A single DMA
            # with a partition-folding rearrange mis-generates descriptors.)
            def relayout(j):
                for s in range(NS):
                    nc.sync.dma_start(
                        out=upk[J * s + j:J * s + j + 1, :],
                        in_=u_sb[j - 1][CH * s:CH * (s + 1), :])

            u1T8 = pool.tile([128, D], f8, tag="u1T8")
            gemm(u_sb[0], y0T, e1_sb)              # u1 = y0 E
            gemm(u_sb[1], y0T8, e2_sb)             # u2' = y0 E2'
            relayout(1)
            transpose_to(None, u_sb[0], dst8=u1T8, scale8=64.0)
            relayout(2)
            gemm(u_sb[2], u1T8, e2_sb)             # u3' = 64 y0 E E2'
            relayout(3)
            gemm(u_sb[3], u1T8, e3_sb)             # u4' = 64 y0 E E3'
            relayout(4)

            g0 = 0
            for blk in (2, 2, 8, 8, 8, 2, 2):
                stage = pool.tile([128, GBLK * D], bf16, tag="stage",
                                  name="stage", bufs=4)
                for gg in range(blk):
                    g = g0 + gg
                    for h in range(2):
                        pc = psum.tile([128, 512], f32, tag="pc", name="pc",
                                       bufs=4)
                        nc.tensor.matmul(
                            pc[:], cm_sb[0:J * NS, :],
                            upk[0:J * NS,
                                (2 * g + h) * 512:(2 * g + h + 1) * 512],
                            start=True, stop=True)
                        dst = stage[:, gg * D + h * 512:gg * D + h * 512 + 512]
                        if h == 0:
                            nc.vector.tensor_copy(dst, pc[:])
                        else:
                            nc.scalar.copy(dst, pc[:])
                for s in range(NS):
                    nc.sync.dma_start(
                        out=out[:, CH * s + g0:CH * s + g0 + blk, :],
                        in_=stage[CH * s:CH * s + NT, 0:blk * D])
                g0 += blk

    nc.compile()
    return nc


def _get_nc():
    nc = _CACHE.get("nc")
    if nc is None:
        nc = _build()
        _CACHE["nc"] = nc
    return nc


def _dopri5_step(y, h, M, b):
    def f(v):
        return v @ M + b
    k1 = f(y)
    k2 = f(y + h * (1.0/5.0) * k1)
    k3 = f(y + h * (3.0/40.0*k1 + 9.0/40.0*k2))
    k4 = f(y + h * (44.0/45.0*k1 - 56.0/15.0*k2 + 32.0/9.0*k3))
    k5 = f(y + h * (19372.0/6561.0*k1 - 25360.0/2187.0*k2
                    + 64448.0/6561.0*k3 - 212.0/729.0*k4))
    k6 = f(y + h * (9017.0/3168.0*k1 - 355.0/33.0*k2 + 46732.0/5247.0*k3
                    + 49.0/176.0*k4 - 5103.0/18656.0*k5))
    return y + h * (35.0/384.0*k1 + 500.0/1113.0*k3 + 125.0/192.0*k4
                    - 2187.0/6784.0*k5 + 11.0/84.0*k6)


def _host_mats(W32, dt):
    """E1 = A - I, E2 = E1^2 for the interval propagator A (f64)."""
    M = W32.T.astype(np.float64)
    S = _dopri5_step(np.eye(D), dt / 2.0, M, 0.0)
    A = S @ S
    E1 = A - np.eye(D)
    E2 = E1 @ E1
    E3 = E2 @ E1
    return E1, E2, E3


def _binom_stationary(jscale):
    from math import comb
    C = np.zeros((32, 128), dtype=np.float64)
    for s in range(NS):
        for j in range(J):
            for tau in range(NT):
                C[J * s + j, 32 * s + tau] = comb(tau + 1, j) / jscale[j]
    return C


def _pow2_scale(E):
    # power-of-2 scale bringing E's std into fp8's sweet spot (~0.25)
    return 2.0 ** int(np.round(np.log2(0.25 / max(E.std(), 1e-300))))


def _fallback(start_embedding, t_eval, W, b):
    M = W.T.astype(np.float64)
    bb = np.asarray(b, dtype=np.float64)
    y = start_embedding.astype(np.float64)
    t = np.asarray(t_eval, dtype=np.float64)
    traj = [y.copy()]
    for k in range(t.shape[0] - 1):
        h = (t[k+1] - t[k]) / 2.0
        for _ in range(2):
            y = _dopri5_step(y, h, M, bb)
        traj.append(y.copy())
    return np.stack(traj).astype(np.float32)


def _kblock(E, bf16):
    # [1024,1024] -> [128, 8*1024] with E_kb[p, 1024 k + n] = E[128 k + p, n]
    return np.ascontiguousarray(
        E.reshape(NB, 128, D).transpose(1, 0, 2).reshape(128, NB * D)
    ).astype(bf16)


def _make_in_maps(y0, t_eval=None, W=None):
    import ml_dtypes
    bf16 = ml_dtypes.bfloat16
    dt = 1.0 if t_eval is None else float(np.asarray(t_eval)[1]
                                          - np.asarray(t_eval)[0])
    import ml_dtypes as mld
    f8 = mld.float8_e4m3
    E1, E2, E3 = _host_mats(W, dt)
    s2 = _pow2_scale(E2)
    s3 = _pow2_scale(E3)
    S1U = 64.0                       # u1 -> fp8 scale hardcoded on device
    e1 = _kblock(E1, bf16)
    e2 = _kblock(E2 * s2, f8)
    e3 = _kblock(E3 * s3, f8)
    cmat = _binom_stationary(
        [1.0, 1.0, s2, S1U * s2, S1U * s3]).astype(bf16)
    maps = []
    for c in range(N_CORES):
        y0c = np.ascontiguousarray(y0[c * ROWS:(c + 1) * ROWS, :]).astype(bf16)
        u0c = np.ascontiguousarray(y0c.reshape(NS, UCOLS))
        maps.append({"y0b": y0c, "e1": e1, "e2": e2, "e3": e3, "u0p": u0c, "cm": cmat})
    return maps


def _assemble(y0, results):
    out = np.empty((T, D, D), dtype=np.float32)
    out[0] = y0
    for c in range(N_CORES):
        dev = results[c]["out"].astype(np.float32)      # [31, 128, 1024]
        out[1:, c * ROWS:(c + 1) * ROWS, :] = dev
    return out


def kernel(start_embedding, t_eval, W, b):
    start_embedding = np.ascontiguousarray(start_embedding, dtype=np.float32)
    W32 = np.ascontiguousarray(W, dtype=np.float32)
    t = np.asarray(t_eval, dtype=np.float64)
    dts = np.diff(t)
    fast_ok = (start_embedding.shape == (D, D) and W32.shape == (D, D)
               and t.shape == (T,) and dts.size > 0
               and np.all(np.abs(dts - dts[0]) <= 1e-12 * abs(dts[0]))
               and not np.any(np.asarray(b)))
    if not fast_ok:
        return _fallback(start_embedding, t_eval, W32, np.asarray(b))

    from concourse.bass_utils import run_bass_kernel_spmd
    nc = _get_nc()
    in_maps = _make_in_maps(start_embedding, t, W32)
    res = run_bass_kernel_spmd(nc, in_maps, list(range(N_CORES)))
    return _assemble(start_embedding, res.results)
